# revision 1
# baseline (speedup 1.0000x reference)
"""Causal self-attention Trainium2 kernel, 8-core SPMD (token-sharded, collective-free).

Model: B=4, T=2048, D=1024, H=16 heads x 64. out = softmax(mask(QK^T/8)) V W_proj^T.

Sharding: 2 cores per batch. Core c handles batch c//2 and the 8 query tiles
(128 tokens each) at real positions t = 2j + (c%2), j=0..7 -- an interleaved
split so the causal work per core is balanced. Each core computes K/V for the
whole batch (modest recompute), attention for its own queries, and the output
projection for its own rows. No collectives; the causal structure difference
between even/odd cores is encoded purely in input data (mask tiles), so the
SPMD program is identical on all cores.

All matmuls run in bf16 (fp32 accumulate). Verified numerics vs the fp32
reference: rel err ~3e-3 (scores are pre-scaled by 1/8 via the Q weights, and
|score| <= ~3.1 so softmax needs no max subtraction).
"""

import os
from contextlib import ExitStack

import numpy as np
import ml_dtypes

import concourse.bass as bass
import concourse.mybir as mybir
import concourse.tile as tile
from concourse import bacc
from concourse.bass_utils import run_bass_kernel_spmd

BF16 = mybir.dt.bfloat16
F32 = mybir.dt.float32
EXP = mybir.ActivationFunctionType.Exp

B, T, D = 4, 2048, 1024
H, DH = 16, 64
NCORES = 8
QT = 8           # q-tiles of 128 per core
KT = 16          # k-tiles of 128 per batch
NPAIR = 8        # head pairs
NEG = -1e9

_cached = {}

if os.environ.get("BASS_LDW_OPT", "") == "1":
    # A/B experiment: let walrus keep/overlap LDWEIGHTS (default path passes
    # --enable-ldw-opt=false)
    from concourse import bass_utils as _bu
    _orig_run_command = _bu.run_command
    def _patched_run_command(argv, **kwargs):
        argv = [a.replace("--enable-ldw-opt=false", "--enable-ldw-opt=true")
                if isinstance(a, str) else a for a in argv]
        return _orig_run_command(argv, **kwargs)
    _bu.run_command = _patched_run_command


def _build_program():
    nc = bacc.Bacc("TRN2", name="causal_attn")

    x_kvT = nc.dram_tensor("x_kvT", [D, T], BF16, kind="ExternalInput")
    x_qT = nc.dram_tensor("x_qT", [D, 1024], BF16, kind="ExternalInput")
    w_qT = nc.dram_tensor("w_qT", [D, D], BF16, kind="ExternalInput")
    w_kT = nc.dram_tensor("w_kT", [D, D], BF16, kind="ExternalInput")
    w_vT = nc.dram_tensor("w_vT", [D, D], BF16, kind="ExternalInput")
    w_pT = nc.dram_tensor("w_pT", [D, D], BF16, kind="ExternalInput")
    maskd = nc.dram_tensor("mask", [128, 256], BF16, kind="ExternalInput")
    outd = nc.dram_tensor("out_T", [D, 1024], F32, kind="ExternalOutput")

    with ExitStack() as ctx:
        tc = ctx.enter_context(tile.TileContext(nc))

        # ---- persistent pools ----
        const = ctx.enter_context(tc.tile_pool(name="const", bufs=1))
        vpool = ctx.enter_context(tc.tile_pool(name="vsb", bufs=1))
        opool = ctx.enter_context(tc.tile_pool(name="osb", bufs=1))
        kpool = ctx.enter_context(tc.tile_pool(name="ksb", bufs=2))
        qpool = ctx.enter_context(tc.tile_pool(name="qsb", bufs=2))
        ppool = ctx.enter_context(tc.tile_pool(name="pex", bufs=6))
        rpool = ctx.enter_context(tc.tile_pool(name="recip", bufs=2))
        bpool = ctx.enter_context(tc.tile_pool(name="bcast", bufs=2))
        tpool = ctx.enter_context(tc.tile_pool(name="otmp", bufs=2))
        drp = ctx.enter_context(tc.tile_pool(name="rscratch", bufs=4, space="DRAM"))
        outsb = ctx.enter_context(tc.tile_pool(name="outsb", bufs=2))
        wpp = ctx.enter_context(tc.tile_pool(name="wp", bufs=1))
        wp = [wpp.tile([128, D], BF16, tag=f"wp{d}", name=f"wp{d}") for d in range(8)]
        mm_ps = ctx.enter_context(tc.tile_pool(name="mm_ps", bufs=4, space="PSUM"))
        st_ps = ctx.enter_context(tc.tile_pool(name="st_ps", bufs=2, space="PSUM"))
        pv_ps = mm_ps

        mask_sb = const.tile([128, 256], BF16)
        nc.sync.dma_start(out=mask_sb[:, :], in_=maskd[:, :])

        V_sb = [vpool.tile([128, H, DH + 1], BF16, tag=f"v{m}", name=f"v{m}") for m in range(KT)]
        O_sb = [opool.tile([128, 1024], BF16, tag=f"o{p}", name=f"o{p}") for p in range(NPAIR)]

        with ExitStack() as s1:
            xkvp = s1.enter_context(tc.tile_pool(name="xkv", bufs=1))
            xqp = s1.enter_context(tc.tile_pool(name="xq", bufs=1))
            wqp = s1.enter_context(tc.tile_pool(name="wq", bufs=1))
            wkp = s1.enter_context(tc.tile_pool(name="wk", bufs=1))
            xkv = [xkvp.tile([128, T], BF16, tag=f"xkv{d}", name=f"xkv{d}") for d in range(8)]
            xq = [xqp.tile([128, 1024], BF16, tag=f"xq{d}", name=f"xq{d}") for d in range(8)]
            wq = [wqp.tile([128, D], BF16, tag=f"wq{d}", name=f"wq{d}") for d in range(8)]
            wk = [wkp.tile([128, D], BF16, tag=f"wk{d}", name=f"wk{d}") for d in range(8)]

            with ExitStack() as s2:
                wvp = s2.enter_context(tc.tile_pool(name="wv", bufs=1))
                wv = [wvp.tile([128, D], BF16, tag=f"wv{d}", name=f"wv{d}") for d in range(8)]
                for d in range(8):
                    nc.sync.dma_start(out=wv[d][:, :], in_=w_vT[128 * d:128 * d + 128, :])
                for cc in range(4):
                    for d in range(8):
                        nc.sync.dma_start(
                            out=xkv[d][:, 512 * cc:512 * cc + 512],
                            in_=x_kvT[128 * d:128 * d + 128, 512 * cc:512 * cc + 512])
                for d in range(8):
                    nc.sync.dma_start(out=wk[d][:, :], in_=w_kT[128 * d:128 * d + 128, :])
                    nc.sync.dma_start(out=wq[d][:, :], in_=w_qT[128 * d:128 * d + 128, :])
                    nc.sync.dma_start(out=xq[d][:, :], in_=x_qT[128 * d:128 * d + 128, :])
                for d in range(8):
                    nc.sync.dma_start(out=wp[d][:, :], in_=w_pT[128 * d:128 * d + 128, :])

                # ---- V (form 1: x stationary), strided into V_sb; the
                # second half is emitted inside pair 0 as dense warm filler ----
                def emit_v(m):
                    for n in range(2):
                        ps = mm_ps.tile([128, 512], F32, tag="ps", name="ps")
                        for d in range(8):
                            nc.tensor.matmul(
                                ps[:, :],
                                lhsT=xkv[d][:, 128 * m:128 * m + 128],
                                rhs=wv[d][:, 512 * n:512 * n + 512],
                                start=(d == 0), stop=(d == 7),
                            )
                        nc.scalar.copy(
                            V_sb[m][:, 8 * n:8 * n + 8, 0:DH],
                            ps[:, :].rearrange("p (h e) -> p h e", h=8),
                        )
                    nc.vector.memset(V_sb[m][:, :, DH:DH + 1], 1.0)

                for m in range(8):
                    emit_v(m)

            # ---- per pair: K/Q projection immediately followed by attention;
            # the dense full-array K/Q bursts keep the PE clock warm and the
            # next pair's K/Q overlaps this pair's ACT/DVE attention tail ----
            def qk(K_t, Q_t, h_off, ki, q0, qw, st_out):
                nc.tensor.matmul(
                    st_out,
                    lhsT=K_t[h_off:h_off + 64, 128 * ki:128 * ki + 128],
                    rhs=Q_t[h_off:h_off + 64, q0:q0 + qw],
                    start=True, stop=True,
                )

            def emit_kq_chunk(p, K_t, Q_t, c):
                # c in 0..3 -> K^T n-chunk c; c in 4..5 -> Q^T n-chunk c-4
                ps = mm_ps.tile([128, 512], F32, tag="ps", name="ps")
                if c < 4:
                    for d in range(8):
                        nc.tensor.matmul(
                            ps[:, :],
                            lhsT=wk[d][:, 128 * p:128 * p + 128],
                            rhs=xkv[d][:, 512 * c:512 * c + 512],
                            start=(d == 0), stop=(d == 7),
                        )
                    nc.vector.tensor_copy(K_t[:, 512 * c:512 * c + 512], ps[:, :])
                else:
                    n = c - 4
                    for d in range(8):
                        nc.tensor.matmul(
                            ps[:, :],
                            lhsT=wq[d][:, 128 * p:128 * p + 128],
                            rhs=xq[d][:, 512 * n:512 * n + 512],
                            start=(d == 0), stop=(d == 7),
                        )
                    nc.vector.tensor_copy(Q_t[:, 512 * n:512 * n + 512], ps[:, :])

            # software-pipelined: pair p's attention interleaves with dense
            # full-array K/Q bursts of pair p+1, keeping the PE clock warm
            KQ = {}
            KQ[0] = (kpool.tile([128, T], BF16, tag="k", name="k0"),
                     qpool.tile([128, 1024], BF16, tag="q", name="q0"))
            for c in range(6):
                emit_kq_chunk(0, KQ[0][0], KQ[0][1], c)

            for p in range(NPAIR):
                K_t, Q_t = KQ[p]
                if p + 1 < NPAIR:
                    KQ[p + 1] = (kpool.tile([128, T], BF16, tag="k", name=f"k{p+1}"),
                                 qpool.tile([128, 1024], BF16, tag="q", name=f"q{p+1}"))

                for J in range(2):
                    # dense K/Q burst for the next pair between attention halves
                    if p + 1 < NPAIR:
                        for c in (0, 1, 2) if J == 0 else (3, 4, 5):
                            emit_kq_chunk(p + 1, KQ[p + 1][0], KQ[p + 1][1], c)
                    if p == 0 and J == 1:
                        for m in range(8, KT):
                            emit_v(m)
                    q0 = 512 * J
                    nbulk = 8 * J
                    nki = nbulk + 8
                    pvs = {}
                    for hi in (0, 1):
                        pvs[hi] = pv_ps.tile([65, 512], F32, tag="ps", name="pv")

                    # One 2-bank st tile per k-tile holds BOTH heads side by
                    # side: the two K=64 QK matmuls are adjacent (concurrent
                    # PE row groups) and a single wide exp covers both heads.
                    # Past the bulk prefix the valid q-blocks form a
                    # contiguous suffix (width 512->128) and the staircase
                    # mask lands on the first 128-col block of each half.
                    for ki in range(nki):
                        e = ki - nbulk
                        qc0 = 0 if e < 0 else 128 * (e // 2)
                        nw = 512 - qc0
                        st = st_ps.tile([128, 2, 512], F32, tag="st", name="st")
                        for hi, h_off in ((0, 0), (1, 64)):
                            qk(K_t, Q_t, h_off, ki, q0 + qc0, nw, st[:, hi, 0:nw])
                        pb = ppool.tile([128, 2, 512], BF16, tag="pb", name="pb")
                        nc.scalar.activation(pb[:, :, 0:nw], st[:, :, 0:nw], EXP)
                        if e >= 0:
                            m0 = 128 * (e & 1)
                            # multiplicative 0/1 causal mask on both heads at
                            # once; the mask operand repeats via a 0-stride dim
                            msrc = mask_sb[:, m0:m0 + 128]
                            mrep = bass.AP(tensor=msrc.tensor, offset=msrc.offset,
                                           ap=[list(msrc.ap[0]), [0, 2], [1, 128]])
                            nc.vector.tensor_mul(pb[:, :, 0:128], pb[:, :, 0:128], mrep)
                        # psum start/stop are bank-granular: start only on
                        # the first matmul into pv, stop only on the last
                        for hi in (0, 1):
                            nc.tensor.matmul(
                                pvs[hi][:, qc0:qc0 + nw],
                                lhsT=V_sb[ki][:, 2 * p + hi, :],
                                rhs=pb[:, hi, 0:nw],
                                start=(ki == 0), stop=(ki == nki - 1),
                            )

                    for hi in (0, 1):
                        pv = pvs[hi]
                        # normalize: sums live in pv row 64; spread them
                        # [128,4] via DRAM so reciprocal runs wide, then
                        # broadcast back across partitions
                        rt = rpool.tile([65, 512], F32)
                        nc.vector.tensor_copy(rt[64:65, :], pv[64:65, :])
                        rd = drp.tile([512], F32, name="rd")
                        nc.gpsimd.dma_start(out=rd[:], in_=rt[64:65, :])
                        rs = rpool.tile([128, 4], F32, name="rs")
                        nc.gpsimd.dma_start(out=rs[:, :], in_=rd.rearrange("(p f) -> p f", p=128))
                        rs2 = rpool.tile([128, 4], F32, name="rs2")
                        nc.vector.reciprocal(rs2[:, :], rs[:, :])
                        rd2 = drp.tile([512], F32, name="rd2")
                        nc.gpsimd.dma_start(out=rd2.rearrange("(p f) -> p f", p=128), in_=rs2[:, :])
                        bc = bpool.tile([64, 512], F32)
                        nc.gpsimd.dma_start(
                            out=bc[:, :],
                            in_=bass.AP(tensor=rd2.tensor, offset=rd2.offset,
                                        ap=[[0, 64]] + list(rd2.ap)),
                        )
                        nc.vector.tensor_mul(
                            O_sb[p][64 * hi:64 * hi + 64, q0:q0 + 512], pv[0:64, :], bc[:, :]
                        )

        # ---- output projection ----
        for m in range(8):
            for n in range(2):
                ps = mm_ps.tile([128, 512], F32, tag="ps", name="ps")
                for p in range(NPAIR):
                    nc.tensor.matmul(
                        ps[:, :],
                        lhsT=wp[p][:, 128 * m:128 * m + 128],
                        rhs=O_sb[p][:, 512 * n:512 * n + 512],
                        start=(p == 0), stop=(p == 7),
                    )
                ob = outsb.tile([128, 512], F32)
                nc.scalar.copy(ob[:, :], ps[:, :])
                nc.sync.dma_start(
                    out=outd[128 * m:128 * m + 128, 512 * n:512 * n + 512],
                    in_=ob[:, :],
                )

    nc.finalize()
    return nc


def _host_inputs(x, W_qkv, W_proj):
    bf = ml_dtypes.bfloat16
    wq = np.ascontiguousarray((W_qkv[0:D] / 8.0).T.astype(bf))
    wk = np.ascontiguousarray(W_qkv[D:2 * D].T.astype(bf))
    wv = np.ascontiguousarray(W_qkv[2 * D:3 * D].T.astype(bf))
    wp = np.ascontiguousarray(W_proj.T.astype(bf))

    kk, qq = np.meshgrid(np.arange(128), np.arange(128), indexing="ij")
    stair = (kk <= qq).astype(np.float32)
    masks = {
        0: np.concatenate([stair, np.zeros((128, 128), np.float32)], axis=1).astype(bf),
        1: np.concatenate([np.ones((128, 128), np.float32), stair], axis=1).astype(bf),
    }

    in_maps = []
    for c in range(NCORES):
        b, fold = c // 2, c % 2
        xT = np.ascontiguousarray(x[b].T.astype(bf))  # [D, T]
        qidx = np.concatenate(
            [np.arange(128 * (2 * j + fold), 128 * (2 * j + fold) + 128) for j in range(QT)]
        )
        in_maps.append({
            "x_kvT": xT,
            "x_qT": np.ascontiguousarray(xT[:, qidx]),
            "w_qT": wq, "w_kT": wk, "w_vT": wv, "w_pT": wp,
            "mask": np.ascontiguousarray(masks[fold]),
        })
    return in_maps


def _run(inputs, trace=False, trace_cores=None):
    if "nc" not in _cached:
        _cached["nc"] = _build_program()
    nc = _cached["nc"]
    in_maps = _host_inputs(inputs["x"], inputs["W_qkv"], inputs["W_proj"])
    res = run_bass_kernel_spmd(
        nc, in_maps, core_ids=list(range(NCORES)),
        trace=trace, trace_cores=trace_cores,
    )
    out = np.zeros((B, T, D), np.float32)
    for c in range(NCORES):
        b, fold = c // 2, c % 2
        oT = res.results[c]["out_T"]  # [D, 1024]
        for j in range(QT):
            t0 = 128 * (2 * j + fold)
            out[b, t0:t0 + 128, :] = oT[:, 128 * j:128 * j + 128].T
    return out, res


def kernel(**inputs) -> np.ndarray:
    out, _ = _run(inputs, trace=os.environ.get("KERNEL_TRACE", "") == "1")
    return out

